# revision 28
# baseline (speedup 1.0000x reference)
"""4-bit comparator (SNN logic-gate network) as a Trainium2 Bass kernel.

Full inputs A, B: [4194304, 4] float32 binary (0/1). Outputs:
  a_gt_b = [N, 1] f32 (1.0 where int4(A) > int4(B), MSB at column 0)
  a_eq_b = [N, 1] f32 (1.0 where all 4 bits equal)

Math: diff = sum_j w_j * (a_j - b_j), w = [8,4,2,1]  (exact in f32)
      gt = diff > 0 ; eq = diff == 0

Sharding: data-parallel over rows across 8 NeuronCores (524288 rows/core).
Per-core layout: flat view of the [S,4] shard tiled as [n, 128, FT]; each
partition holds FT/4 consecutive rows (bits along the free dim, MSB first).

Per tile (DVE only; memory-bound kernel):
  d  = A - B                      (tensor_tensor, dense)
  q  = 2*d[even] + d[odd]         (scalar_tensor_tensor madd, stride 2)
  df = 4*q[even] + q[odd]         (scalar_tensor_tensor madd, stride 2)
  gt = df > 0 ; eq = df == 0      (tensor_scalar is_gt / is_equal)
"""

import functools
import sys

sys.path.insert(0, "/opt/trn_rl_repo")

import numpy as np

import concourse.bass as bass
import concourse.tile as tile
from concourse import bacc, mybir
from concourse.alu_op_type import AluOpType
from concourse.bass_utils import run_bass_kernel_spmd

P = 128
N_CORES = 8
FT_DEFAULT = 4096  # free-dim f32 elements per partition per tile (2 MiB DMAs)


def build_nc(S: int, FT: int = FT_DEFAULT, reps: int = 1, internal_out: bool = False,
             bufs_io: int = 3, bufs_tmp: int = 1, mode: str = "full",
             queues: str = "split", inplace: bool = False, staggered: bool = False,
             batch_writes: bool = False):
    """Build the single-core Bass program for an [S, 4] shard.

    reps > 1 repeats the whole pipeline in-NEFF (benchmarking only).
    internal_out=True makes GT/EQ internal DRAM (same HBM traffic) with a
    4-byte ExternalOutput instead, so benchmark calls fetch ~nothing.
    """
    R = FT // 4          # rows per partition per tile
    rows_per_tile = P * R
    assert S % rows_per_tile == 0, (S, rows_per_tile)
    n_tiles = S // rows_per_tile
    f32 = mybir.dt.float32

    nc = bacc.Bacc("TRN2", target_bir_lowering=False, debug=False)
    out_kind = "Internal" if internal_out else "ExternalOutput"
    if mode == "ab":
        AB = nc.dram_tensor("AB", [2 * S, 4], f32, kind="ExternalInput").ap()
        ABv = AB.rearrange("(h n p r) j -> n p h (r j)", h=2, p=P, r=R)
        Av = Bv = None
    else:
        A = nc.dram_tensor("A", [S, 4], f32, kind="ExternalInput").ap()
        B = nc.dram_tensor("B", [S, 4], f32, kind="ExternalInput").ap()
        Av = A.rearrange("(n p r) j -> n p (r j)", p=P, r=R)
        Bv = B.rearrange("(n p r) j -> n p (r j)", p=P, r=R)
    GT = nc.dram_tensor("GT", [S, 1], f32, kind=out_kind).ap()
    EQ = nc.dram_tensor("EQ", [S, 1], f32, kind=out_kind).ap()
    GTv = GT.rearrange("(n p r) j -> n p (r j)", p=P, r=R)
    EQv = EQ.rearrange("(n p r) j -> n p (r j)", p=P, r=R)

    import contextlib

    with tile.TileContext(nc) as tc:
        with (
            tc.tile_pool(name="io", bufs=bufs_io) as io,
            tc.tile_pool(name="tmp", bufs=bufs_tmp) as tmp,
            tc.tile_pool(name="sm", bufs=max(bufs_tmp, 3)) as sm,
            tc.tile_pool(name="outp", bufs=(S // (P * (FT // 4))) if batch_writes else 3) as outp,
        ):
            if queues == "split":
                eng_a, eng_b, eng_gt, eng_eq = nc.sync, nc.scalar, nc.sync, nc.scalar
            elif queues == "gpcmp":
                eng_a, eng_b, eng_gt, eng_eq = nc.sync, nc.scalar, nc.sync, nc.scalar
            elif queues == "swap":
                eng_a, eng_b, eng_gt, eng_eq = nc.sync, nc.scalar, nc.scalar, nc.sync
            elif queues == "split2":
                eng_a, eng_b, eng_gt, eng_eq = nc.sync, nc.scalar, nc.sync, nc.scalar
            elif queues == "inout":
                eng_a, eng_b, eng_gt, eng_eq = nc.sync, nc.sync, nc.scalar, nc.scalar
            elif queues == "gpout":
                eng_a, eng_b, eng_gt, eng_eq = nc.sync, nc.scalar, nc.gpsimd, nc.gpsimd
            else:
                eng_a = eng_b = eng_gt = eng_eq = nc.sync
            loop_cm = (
                tc.For_i(0, reps, 1, staggered_reset=staggered)
                if reps > 1 else contextlib.nullcontext()
            )
            with loop_cm:
                deferred = []
                for t in range(n_tiles):
                    if mode == "ab":
                        tab = io.tile([P, 2 * FT], f32, tag="tab")
                        (eng_a if t % 2 == 0 else eng_b).dma_start(tab[:], ABv[t])
                        ta_ap, tb_ap = tab[:, :FT], tab[:, FT:]
                    else:
                        ta = io.tile([P, FT], f32, tag="ta")
                        tb = io.tile([P, FT], f32, tag="tb")
                        ta_ap, tb_ap = ta[:], tb[:]
                    if mode == "ab":
                        pass
                    elif queues == "split2":
                        h = FT // 2
                        eng_a.dma_start(ta[:, :h], Av[t, :, :h])
                        eng_b.dma_start(ta[:, h:], Av[t, :, h:])
                        eng_b.dma_start(tb[:, :h], Bv[t, :, :h])
                        eng_a.dma_start(tb[:, h:], Bv[t, :, h:])
                    else:
                        eng_a.dma_start(ta[:], Av[t])
                        eng_b.dma_start(tb[:], Bv[t])

                    sentinel = ta_ap
                    if mode == "read_only":
                        # force both loads without bulk HBM writes
                        ro = sm.tile([P, 1], f32, tag="ro")
                        nc.vector.tensor_scalar(
                            ro[:], ta_ap[:, 0:1], 0.0, None, AluOpType.add)
                        ro2 = sm.tile([P, 1], f32, tag="ro2")
                        nc.vector.tensor_scalar(
                            ro2[:], tb_ap[:, 0:1], 0.0, None, AluOpType.add)
                        continue
                    if mode == "dma_only":
                        eng_gt.dma_start(GTv[t], ta_ap[:, 0:R])
                        eng_eq.dma_start(EQv[t], tb_ap[:, 0:R])
                        continue

                    if inplace:
                        d_ap = ta_ap
                        nc.vector.tensor_tensor(d_ap, ta_ap, tb_ap, AluOpType.subtract)
                        dv = d_ap.rearrange("p (k two) -> p k two", two=2)
                        q_ap = tb[:, : FT // 2]
                        nc.vector.scalar_tensor_tensor(
                            q_ap, dv[:, :, 0], 2.0, dv[:, :, 1],
                            AluOpType.mult, AluOpType.add,
                        )
                        qv = q_ap.rearrange("p (k two) -> p k two", two=2)
                    else:
                        d = tmp.tile([P, FT], f32, tag="d")
                        nc.vector.tensor_tensor(d[:], ta_ap, tb_ap, AluOpType.subtract)

                        dv = d[:].rearrange("p (k two) -> p k two", two=2)
                        q = tmp.tile([P, FT // 2], f32, tag="q")
                        nc.vector.scalar_tensor_tensor(
                            q[:], dv[:, :, 0], 2.0, dv[:, :, 1],
                            AluOpType.mult, AluOpType.add,
                        )

                        qv = q[:].rearrange("p (k two) -> p k two", two=2)
                    df = tmp.tile([P, R], f32, tag="df")
                    nc.vector.scalar_tensor_tensor(
                        df[:], qv[:, :, 0], 4.0, qv[:, :, 1],
                        AluOpType.mult, AluOpType.add,
                    )

                    cmp_eng = nc.gpsimd if queues == "gpcmp" else nc.vector
                    gt_t = outp.tile([P, R], f32, tag="gt")
                    cmp_eng.tensor_scalar(gt_t[:], df[:], 0.0, None, AluOpType.is_gt)
                    eq_t = outp.tile([P, R], f32, tag="eq")
                    cmp_eng.tensor_scalar(eq_t[:], df[:], 0.0, None, AluOpType.is_equal)

                    if batch_writes:
                        deferred.append((t, gt_t, eq_t))
                    else:
                        eng_gt.dma_start(GTv[t], gt_t[:])
                        eng_eq.dma_start(EQv[t], eq_t[:])
                for t, g, e in deferred:
                    eng_gt.dma_start(GTv[t], g[:])
                    eng_eq.dma_start(EQv[t], e[:])
            if internal_out:
                OUT = nc.dram_tensor("OUT", [1, 1], f32, kind="ExternalOutput").ap()
                src = eq_t if mode == "full" else sentinel
                nc.sync.dma_start(OUT[:], src[0:1, 0:1])
    nc.compile()
    return nc


@functools.lru_cache(maxsize=None)
def _get_nc(S: int, FT: int):
    return build_nc(S, FT)


def kernel(A: np.ndarray, B: np.ndarray):
    A = np.asarray(A, dtype=np.float32)
    B = np.asarray(B, dtype=np.float32)
    N = A.shape[0]

    # Pad rows so each core gets a whole number of [128, FT] tiles.
    chunk = N_CORES * P * (FT_DEFAULT // 4)
    N_pad = -(-N // chunk) * chunk
    if N_pad != N:
        pad = ((0, N_pad - N), (0, 0))
        A = np.pad(A, pad)
        B = np.pad(B, pad)
    S = N_pad // N_CORES

    nc = _get_nc(S, FT_DEFAULT)
    in_maps = [
        {
            "A": np.ascontiguousarray(A[i * S : (i + 1) * S]),
            "B": np.ascontiguousarray(B[i * S : (i + 1) * S]),
        }
        for i in range(N_CORES)
    ]
    res = run_bass_kernel_spmd(nc, in_maps, list(range(N_CORES)))
    gt = np.concatenate([r["GT"] for r in res.results], axis=0)[:N]
    eq = np.concatenate([r["EQ"] for r in res.results], axis=0)[:N]
    return gt, eq
